# revision 1
# baseline (speedup 1.0000x reference)
"""Trainium2 Bass kernel for nn_CASAtt_MultiHead_v1 (CAS attention block).

Reference computation (per sample):
    qkv = 1x1 conv (qkv_w) -> q, k, v                        [512, 56, 56] each
    q <- SE(dwconv3x3(q, sq_w, sq_b))   (per-head squeeze-excite)
    k <- SE(dwconv3x3(k, sk_w, sk_b))
    out = proj(dwconv3x3(q + k, dwc_w, dwc_b) * v) + proj_b + x

Distribution: data-parallel over batch, 2 samples per NeuronCore x 8 cores.

Layout: channels on partitions, 4 chunks of 128 (chunk == SE head).
GEMMs run in bf16 (fp32 PSUM accumulate); fp32 matmuls on trn2 run in
LOW_HIGH mode at ~3x the cost, so everything streaming through the PE is
bf16.  Depthwise convs: 9 taps, applied either as diagonal-matrix matmuls
accumulated in PSUM (TensorE), or as fused scalar-MAC chains
(scalar_tensor_tensor) on VectorE over *contiguous* padded-flat slices so
the DVE 2x bf16 perf mode engages where alignment allows (strided views
drop it to 1x).  The conv domain is zero-padded HPxWP with WP=59 so that
5 of 9 tap offsets have even element parity (bf16 4-byte alignment for
the DVE perf mode).  Conv outputs computed over full padded rows produce
garbage only in pad columns, which are never read.  SE average-pool is
fused into the conv1 drain via accum_out.  Since depthwise conv and the
SE scale are per-channel linear, m = s_q*dwq + s_k*dwk is built after
both branches and a single third conv runs on m.  Mixed-dtype
tensor-tensor DVE ops (psum f32 + bf16 operand) produce NaN on hardware
(fine in CoreSim) -- every tensor-tensor-class op here keeps both tensor
operands the same dtype.
"""

import numpy as np

DIM = 512
NH = 4
HD = 128
HD4 = 32
B, H_FULL, W = 16, 56, 56
N_CORES = 8

TAPS = [(dy, dx) for dy in (-1, 0, 1) for dx in (-1, 0, 1)]


def default_cfg():
    return dict(
        b_local=B // N_CORES,
        H=H_FULL,
        rows_per_tile=8,
        conv_bf16=True,
        gemm_bf16=True,
        # engine per (branch, oc) for conv1:  'pe' | 'dve'
        conv1_assign={(br, oc): 'pe' for br in (0, 1) for oc in range(NH)},
        repeat=1,
    )


def build_nc(cfg):
    """Build + compile the Bacc program for one core (SPMD across 8)."""
    import concourse.bass as bass
    import concourse.mybir as mybir
    import concourse.tile as tile
    from concourse import bacc
    from contextlib import ExitStack

    f32 = mybir.dt.float32
    bf16 = mybir.dt.bfloat16
    cdt = bf16 if cfg['conv_bf16'] else f32
    gdt = bf16 if cfg['gemm_bf16'] else f32

    BL = cfg['b_local']
    H = cfg['H']
    TH = cfg['rows_per_tile']
    NT = H // TH
    assert NT * TH == H
    TN = TH * W
    HP, WP = H + 2, W + 2
    PADN = HP * WP
    TPAD = TH * WP
    AF = mybir.ActivationFunctionType
    AL = mybir.AluOpType
    # DVE tap order: even-parity offsets first (2x mode), odd-parity last;
    # the final op is strided (1x regardless), so give it an odd tap
    evens = [j for j, (dy, dx) in enumerate(TAPS) if (dy * WP + dx) % 2 == 0]
    odds = [j for j, (dy, dx) in enumerate(TAPS) if (dy * WP + dx) % 2]
    dve_tap_order = evens + odds if odds else list(range(9))

    nc = bacc.Bacc("TRN2", target_bir_lowering=False, debug=False,
                   enable_asserts=False, num_devices=N_CORES)

    # ---------------- DRAM I/O ----------------
    x_d = nc.dram_tensor("x", [BL, DIM, H, W], gdt, kind="ExternalInput").ap()
    out_d = nc.dram_tensor("out", [BL, DIM, H, W], f32, kind="ExternalOutput").ap()
    wq_d = nc.dram_tensor("wq_t", [DIM, DIM], gdt, kind="ExternalInput").ap()
    wk_d = nc.dram_tensor("wk_t", [DIM, DIM], gdt, kind="ExternalInput").ap()
    wv_d = nc.dram_tensor("wv_t", [DIM, DIM], gdt, kind="ExternalInput").ap()
    wp_d = nc.dram_tensor("proj_t", [DIM, DIM], gdt, kind="ExternalInput").ap()
    dg1_d = [nc.dram_tensor(n, [NH, 9, HD, HD], cdt, kind="ExternalInput").ap()
             for n in ("diag1q", "diag1k")]
    dg2_d = nc.dram_tensor("diag2", [NH, 9, HD, HD], cdt, kind="ExternalInput").ap()
    wv1_d = [nc.dram_tensor(n, [NH, HD, 9], f32, kind="ExternalInput").ap()
             for n in ("wvec1q", "wvec1k")]
    b1_d = [nc.dram_tensor(n, [DIM, 1], f32, kind="ExternalInput").ap()
            for n in ("sq_b", "sk_b")]
    dwcb_d = nc.dram_tensor("dwc_b", [DIM, 1], f32, kind="ExternalInput").ap()
    projb_d = nc.dram_tensor("proj_b", [DIM, 1], f32, kind="ExternalInput").ap()
    sew1_d = [nc.dram_tensor(n, [NH, HD, HD4], f32, kind="ExternalInput").ap()
              for n in ("se_w1q", "se_w1k")]
    seb1_d = [nc.dram_tensor(n, [NH, HD4, 1], f32, kind="ExternalInput").ap()
              for n in ("se_b1q", "se_b1k")]
    sew2_d = [nc.dram_tensor(n, [NH, HD4, HD], f32, kind="ExternalInput").ap()
              for n in ("se_w2q", "se_w2k")]
    seb2_d = [nc.dram_tensor(n, [NH, HD, 1], f32, kind="ExternalInput").ap()
              for n in ("se_b2q", "se_b2k")]

    with tile.TileContext(nc) as tc, ExitStack() as ctx:
        const = ctx.enter_context(tc.tile_pool(name="const", bufs=1))
        big = ctx.enter_context(tc.tile_pool(name="big", bufs=1))
        wpool = ctx.enter_context(tc.tile_pool(name="wpool", bufs=1))
        xpool = ctx.enter_context(tc.tile_pool(name="xpool", bufs=2))
        vpool = ctx.enter_context(tc.tile_pool(name="vpool", bufs=2))
        o2pool = ctx.enter_context(tc.tile_pool(name="o2pool", bufs=2))
        otpool = ctx.enter_context(tc.tile_pool(name="otpool", bufs=2))
        dgpool = ctx.enter_context(tc.tile_pool(name="dgpool", bufs=1))
        statpool = ctx.enter_context(tc.tile_pool(name="statpool", bufs=2))
        mmpool = ctx.enter_context(tc.tile_pool(name="mmpool", bufs=5, space="PSUM"))
        sepool = ctx.enter_context(tc.tile_pool(name="sepool", bufs=2, space="PSUM"))

        # ---------- persistent SBUF ----------
        # padded conv-domain buffers; 2-elem slop so padded-space tap reads
        # (offsets -WP-1 .. +WP+1) stay in bounds
        qpad = [big.tile([HD, PADN + 2], cdt, name=f"qpad{c}") for c in range(NH)]
        kpad = [big.tile([HD, PADN + 2], cdt, name=f"kpad{c}") for c in range(NH)]
        dwq = [big.tile([HD, PADN + 2], cdt, name=f"dwq{c}") for c in range(NH)]
        dwk = [big.tile([HD, PADN + 2], cdt, name=f"dwk{c}") for c in range(NH)]

        def pad3(t):
            return t[:, 1:1 + PADN].rearrange("p (h w) -> p h w", w=WP)

        qpad3, kpad3 = [pad3(t) for t in qpad], [pad3(t) for t in kpad]
        dwq3, dwk3 = [pad3(t) for t in dwq], [pad3(t) for t in dwk]

        # persistent DVE-conv accumulator rotation buffers (pad cells may hold
        # stale garbage between uses; only interior cells are ever consumed)
        acc_g = [big.tile([HD, PADN + 2], cdt, name=f"accg{i}") for i in range(3)]

        for tt in qpad + kpad + dwq + dwk + acc_g:
            nc.vector.memset(tt, 0.0)

        # small constants
        bias1 = [[const.tile([HD, 1], f32, name=f"b1_{br}_{c}") for c in range(NH)]
                 for br in range(2)]
        dwcb = [const.tile([HD, 1], f32, name=f"dwcb{c}") for c in range(NH)]
        projb = [const.tile([HD, 1], f32, name=f"projb{c}") for c in range(NH)]
        for c in range(NH):
            sl = slice(c * HD, (c + 1) * HD)
            for br in range(2):
                nc.sync.dma_start(bias1[br][c], b1_d[br][sl])
            nc.sync.dma_start(dwcb[c], dwcb_d[sl])
            nc.sync.dma_start(projb[c], projb_d[sl])
        sew1 = [[const.tile([HD, HD4], f32, name=f"sew1_{br}_{c}") for c in range(NH)]
                for br in range(2)]
        seb1 = [[const.tile([HD4, 1], f32, name=f"seb1_{br}_{c}") for c in range(NH)]
                for br in range(2)]
        sew2 = [[const.tile([HD4, HD], f32, name=f"sew2_{br}_{c}") for c in range(NH)]
                for br in range(2)]
        seb2 = [[const.tile([HD, 1], f32, name=f"seb2_{br}_{c}") for c in range(NH)]
                for br in range(2)]
        wvec1 = [[const.tile([HD, 9], f32, name=f"wvec_{br}_{c}") for c in range(NH)]
                 for br in range(2)]
        for br in range(2):
            for c in range(NH):
                nc.sync.dma_start(sew1[br][c], sew1_d[br][c])
                nc.sync.dma_start(seb1[br][c], seb1_d[br][c])
                nc.sync.dma_start(sew2[br][c], sew2_d[br][c])
                nc.sync.dma_start(seb2[br][c], seb2_d[br][c])
                nc.sync.dma_start(wvec1[br][c], wv1_d[br][c])

        def taps_flat_tile(tbuf, r0):
            """9 contiguous slices (full padded rows) for padded-space conv
            over output padded rows r0+1..r0+TH (tile granularity, PE)."""
            base = 1 + (r0 + 1) * WP
            return [tbuf[:, base + dy * WP + dx: base + dy * WP + dx + TPAD]
                    for (dy, dx) in TAPS]

        def tap_bounds(j):
            """Whole-chunk padded-flat bounds for tap j: covers padded rows
            1..H (all interior rows; top/bottom pad rows excluded so reads
            stay within the slop), start/count adjusted to even element
            parity.  Only pad cells are dropped by the adjustments."""
            dy, dx = TAPS[j]
            delta = dy * WP + dx
            lo, cnt = WP, H * WP
            if (1 + lo + delta) % 2:
                lo, cnt = lo + 1, cnt - 1
            if cnt % 2:
                cnt -= 1
            return lo, cnt, delta

        def emit_body(rep):
            sfx = f"_r{rep}" if cfg['repeat'] > 1 else ""
            s_scale = [[[None] * NH for _ in range(2)] for _ in range(BL)]

            def phase1a(b):
                # ============ PHASE 1a: q and k GEMMs ============
                for br in range(2):
                    w_d = wq_d if br == 0 else wk_d
                    p3 = qpad3 if br == 0 else kpad3
                    w_sb = []
                    for kc in range(NH):
                        row = []
                        for oc in range(NH):
                            wt = wpool.tile([HD, HD], gdt, tag=f"wA{kc}_{oc}",
                                            name=f"wA{kc}_{oc}_b{b}_{br}{sfx}")
                            nc.sync.dma_start(wt, w_d[kc * HD:(kc + 1) * HD,
                                                      oc * HD:(oc + 1) * HD])
                            row.append(wt)
                        w_sb.append(row)
                    for t in range(NT):
                        r0 = t * TH
                        xt = []
                        for kc in range(NH):
                            xx = xpool.tile([HD, TN], gdt, tag=f"xt{kc}",
                                            name=f"xt{kc}_b{b}_{br}_{t}{sfx}")
                            nc.sync.dma_start(
                                xx.rearrange("p (h w) -> p h w", w=W),
                                x_d[b, kc * HD:(kc + 1) * HD, r0:r0 + TH, :])
                            xt.append(xx)
                        for oc in range(NH):
                            ps = mmpool.tile([HD, TN], f32, tag="mm",
                                             name=f"g{b}_{br}_{t}_{oc}{sfx}")
                            for kc in range(NH):
                                nc.tensor.matmul(ps, w_sb[kc][oc], xt[kc],
                                                 start=(kc == 0),
                                                 stop=(kc == NH - 1))
                            nc.scalar.copy(
                                p3[oc][:, 1 + r0:1 + r0 + TH, 1:1 + W],
                                ps.rearrange("p (h w) -> p h w", w=W))

            def phase1b(b, br):
                # ============ PHASE 1b: conv1 + SE for one branch ============
                if True:
                    srcb = qpad if br == 0 else kpad
                    src3 = qpad3 if br == 0 else kpad3
                    dstb = dwq if br == 0 else dwk
                    dst3 = dwq3 if br == 0 else dwk3
                    for oc in range(NH):
                        eng = cfg['conv1_assign'][(br, oc)]
                        stats = statpool.tile([HD, NT], f32, tag="stats",
                                              name=f"st{b}_{br}_{oc}{sfx}")
                        if eng == 'pe':
                            dgs = []
                            for j in range(9):
                                dg = dgpool.tile([HD, HD], cdt, tag=f"dg{j}",
                                                 name=f"dg{j}_{b}_{br}_{oc}{sfx}")
                                nc.sync.dma_start(dg, dg1_d[br][oc, j])
                                dgs.append(dg)
                            for t in range(NT):
                                r0 = t * TH
                                ps = mmpool.tile([HD, TPAD], f32, tag="mm",
                                                 name=f"c1{b}_{br}_{t}_{oc}{sfx}")
                                for j, v in enumerate(taps_flat_tile(srcb[oc], r0)):
                                    nc.tensor.matmul(ps, dgs[j], v,
                                                     start=(j == 0), stop=(j == 8))
                                nc.scalar.activation(
                                    dst3[oc][:, 1 + r0:1 + r0 + TH, 1:1 + W],
                                    ps.rearrange("p (h w) -> p h w",
                                                 w=WP)[:, :, 1:1 + W],
                                    AF.Identity, bias=bias1[br][oc],
                                    accum_out=stats[:, t:t + 1])
                            pooled_w = NT
                        else:
                            # hybrid DVE conv: even-parity taps as 4x
                            # tensor_scalar scaled copies, odd taps as ACT
                            # scaled copies ('dva') or 1x STT ('dve'); all
                            # accumulated by 2x tensor_tensor over a fixed
                            # 4-byte-aligned canonical range.
                            wvec = wvec1[br][oc]
                            clo, ccnt = WP + 1, H * WP - 2
                            order = dve_tap_order
                            j0 = order[0]            # an even tap
                            lo, cnt, delta = tap_bounds(j0)
                            cur, nxt, tmp = 0, 1, 2   # acc_g roles
                            nc.vector.tensor_scalar(
                                acc_g[cur][:, 1 + lo:1 + lo + cnt],
                                srcb[oc][:, 1 + lo + delta:1 + lo + delta + cnt],
                                wvec[:, j0:j0 + 1], bias1[br][oc],
                                AL.mult, AL.add)
                            for jj in order[1:8]:
                                lo, cnt, delta = tap_bounds(jj)
                                even = (delta % 2 == 0)
                                if even or eng == 'dva':
                                    src_sl = srcb[oc][:, 1 + lo + delta:
                                                      1 + lo + delta + cnt]
                                    t_sl = acc_g[tmp][:, 1 + lo:1 + lo + cnt]
                                    if even:
                                        nc.vector.tensor_scalar(
                                            t_sl, src_sl, wvec[:, jj:jj + 1],
                                            None, AL.mult)
                                    else:
                                        nc.scalar.activation(
                                            t_sl, src_sl, AF.Copy,
                                            scale=wvec[:, jj:jj + 1])
                                    nc.vector.tensor_tensor(
                                        acc_g[nxt][:, 1 + clo:1 + clo + ccnt],
                                        acc_g[cur][:, 1 + clo:1 + clo + ccnt],
                                        acc_g[tmp][:, 1 + clo:1 + clo + ccnt],
                                        AL.add)
                                    cur, nxt, tmp = nxt, tmp, cur
                                else:
                                    nc.vector.scalar_tensor_tensor(
                                        acc_g[nxt][:, 1 + lo:1 + lo + cnt],
                                        srcb[oc][:, 1 + lo + delta:
                                                 1 + lo + delta + cnt],
                                        wvec[:, jj:jj + 1],
                                        acc_g[cur][:, 1 + lo:1 + lo + cnt],
                                        AL.mult, AL.add)
                                    cur, nxt = nxt, cur
                            # last tap (odd parity): strided interior finalize
                            j8 = order[8]
                            dy, dx = TAPS[j8]
                            if eng == 'dva':
                                # ACT scaled copy of the last tap, 2x TT
                                # accumulate, ACT strided store + pooling
                                lo, cnt, delta = tap_bounds(j8)
                                t_sl = acc_g[tmp][:, 1 + lo:1 + lo + cnt]
                                nc.scalar.activation(
                                    t_sl,
                                    srcb[oc][:, 1 + lo + delta:
                                             1 + lo + delta + cnt],
                                    AF.Copy, scale=wvec[:, j8:j8 + 1])
                                nc.vector.tensor_tensor(
                                    acc_g[nxt][:, 1 + clo:1 + clo + ccnt],
                                    acc_g[cur][:, 1 + clo:1 + clo + ccnt],
                                    acc_g[tmp][:, 1 + clo:1 + clo + ccnt],
                                    AL.add)
                                nc.scalar.activation(
                                    dst3[oc][:, 1:1 + H, 1:1 + W],
                                    pad3(acc_g[nxt])[:, 1:1 + H, 1:1 + W],
                                    AF.Identity, bias=0.0,
                                    accum_out=stats[:, 0:1])
                            else:
                                nc.vector.scalar_tensor_tensor(
                                    dst3[oc][:, 1:1 + H, 1:1 + W],
                                    src3[oc][:, 1 + dy:1 + dy + H,
                                             1 + dx:1 + dx + W],
                                    wvec[:, j8:j8 + 1],
                                    pad3(acc_g[cur])[:, 1:1 + H, 1:1 + W],
                                    AL.mult, AL.add,
                                    accum_out=stats[:, 0:1])
                            pooled_w = 1
                        # ---- SE for this head ----
                        pooled = const.tile([HD, 1], f32, tag="pooled",
                                            bufs=4, name=f"pool{b}_{br}_{oc}{sfx}")
                        nc.vector.tensor_reduce(pooled, stats[:, 0:pooled_w],
                                                mybir.AxisListType.X, AL.add)
                        ps1 = sepool.tile([HD4, 1], f32, tag="se",
                                          name=f"se1_{b}_{br}_{oc}{sfx}")
                        nc.tensor.matmul(ps1, sew1[br][oc], pooled,
                                         start=True, stop=True)
                        hvec = const.tile([HD4, 1], f32, tag="hvec", bufs=4,
                                          name=f"h{b}_{br}_{oc}{sfx}")
                        nc.scalar.activation(hvec, ps1, AF.Relu,
                                             bias=seb1[br][oc])
                        ps2 = sepool.tile([HD, 1], f32, tag="se",
                                          name=f"se2_{b}_{br}_{oc}{sfx}")
                        nc.tensor.matmul(ps2, sew2[br][oc], hvec,
                                         start=True, stop=True)
                        s_sb = const.tile([HD, 1], f32, tag="s_scale", bufs=16,
                                          name=f"s{b}_{br}_{oc}{sfx}")
                        nc.scalar.activation(s_sb, ps2, AF.Sigmoid,
                                             bias=seb2[br][oc])
                        s_scale[b][br][oc] = s_sb

            def phase15(b):
                # ======== PHASE 1.5: m = s_q*dwq + s_k*dwk (into dwk) ======
                for oc in range(NH):
                    scr = acc_g[oc % 2]
                    nc.vector.tensor_scalar(scr, dwk[oc], s_scale[b][1][oc],
                                            None, AL.mult)
                    nc.vector.scalar_tensor_tensor(dwk[oc], dwq[oc],
                                                   s_scale[b][0][oc], scr,
                                                   AL.mult, AL.add)

            def phase2(b):
                # ================= PHASE 2 =================
                wv_sb, wp_sb = [], []
                for kc in range(NH):
                    rowv, rowp = [], []
                    for oc in range(NH):
                        wt = wpool.tile([HD, HD], gdt, tag=f"wV{kc}_{oc}",
                                        name=f"wV{kc}_{oc}_b{b}{sfx}")
                        nc.sync.dma_start(wt, wv_d[kc * HD:(kc + 1) * HD,
                                                   oc * HD:(oc + 1) * HD])
                        rowv.append(wt)
                        wt2 = wpool.tile([HD, HD], gdt, tag=f"wB{kc}_{oc}",
                                         name=f"wP{kc}_{oc}_b{b}{sfx}")
                        nc.sync.dma_start(wt2, wp_d[kc * HD:(kc + 1) * HD,
                                                    oc * HD:(oc + 1) * HD])
                        rowp.append(wt2)
                    wv_sb.append(rowv)
                    wp_sb.append(rowp)
                dg2 = []
                for oc in range(NH):
                    row = []
                    for j in range(9):
                        dg = dgpool.tile([HD, HD], cdt, tag=f"dg2_{oc}_{j}",
                                         name=f"dg2_{oc}_{j}_b{b}{sfx}")
                        nc.sync.dma_start(dg, dg2_d[oc, j])
                        row.append(dg)
                    dg2.append(row)
                for t in range(NT):
                    r0 = t * TH
                    xt = []
                    for kc in range(NH):
                        xx = xpool.tile([HD, TN], gdt, tag=f"xt{kc}",
                                        name=f"x2_{kc}_b{b}_{t}{sfx}")
                        nc.sync.dma_start(
                            xx.rearrange("p (h w) -> p h w", w=W),
                            x_d[b, kc * HD:(kc + 1) * HD, r0:r0 + TH, :])
                        xt.append(xx)
                    v_sb = []
                    for oc in range(NH):
                        ps = mmpool.tile([HD, TN], f32, tag="mm",
                                         name=f"v{b}_{t}_{oc}{sfx}")
                        for kc in range(NH):
                            nc.tensor.matmul(ps, wv_sb[kc][oc], xt[kc],
                                             start=(kc == 0), stop=(kc == NH - 1))
                        vv = vpool.tile([HD, TN], gdt, tag=f"vt{oc}",
                                        name=f"vt{oc}_b{b}_{t}{sfx}")
                        nc.scalar.copy(vv, ps)
                        v_sb.append(vv)
                    o2 = []
                    for oc in range(NH):
                        ps = mmpool.tile([HD, TPAD], f32, tag="mm",
                                         name=f"c2{b}_{t}_{oc}{sfx}")
                        for j, v in enumerate(taps_flat_tile(dwk[oc], r0)):
                            nc.tensor.matmul(ps, dg2[oc][j], v,
                                             start=(j == 0), stop=(j == 8))
                        c2t = o2pool.tile([HD, TN], gdt, tag="c2t", bufs=3,
                                          name=f"c2t_{oc}_b{b}_{t}{sfx}")
                        nc.scalar.activation(
                            c2t.rearrange("p (h w) -> p h w", w=W),
                            ps.rearrange("p (h w) -> p h w", w=WP)[:, :, 1:1 + W],
                            AF.Identity, bias=dwcb[oc])
                        oo = o2pool.tile([HD, TN], gdt, tag=f"o2_{oc}",
                                         name=f"o2_{oc}_b{b}_{t}{sfx}")
                        nc.vector.tensor_mul(oo, c2t, v_sb[oc])
                        o2.append(oo)
                    for oc in range(NH):
                        ps = mmpool.tile([HD, TN], f32, tag="mm",
                                         name=f"p{b}_{t}_{oc}{sfx}")
                        for kc in range(NH):
                            nc.tensor.matmul(ps, wp_sb[kc][oc], o2[kc],
                                             start=(kc == 0), stop=(kc == NH - 1))
                        # residual: convert x tile to f32 on ACT, then
                        # (proj + proj_b) + x with all-f32 operands
                        xc = otpool.tile([HD, TN], f32, tag=f"xc{oc}",
                                         name=f"xc{oc}_b{b}_{t}{sfx}")
                        nc.scalar.copy(xc, xt[oc])
                        ot = otpool.tile([HD, TN], f32, tag=f"ot{oc}",
                                         name=f"ot{oc}_b{b}_{t}{sfx}")
                        nc.vector.scalar_tensor_tensor(ot, ps, projb[oc],
                                                       xc, AL.add, AL.add)
                        nc.sync.dma_start(
                            out_d[b, oc * HD:(oc + 1) * HD, r0:r0 + TH, :],
                            ot.rearrange("p (h w) -> p h w", w=W))

            if BL == 2 and cfg.get('interleave', False):
                # interleave samples so DVE conv work overlaps PE phases
                phase1a(0)
                phase1b(0, 0)
                phase1b(0, 1)
                phase1a(1)
                phase15(0)
                phase1b(1, 0)
                phase2(0)
                phase1b(1, 1)
                phase15(1)
                phase2(1)
            else:
                for b in range(BL):
                    phase1a(b)
                    phase1b(b, 0)
                    phase1b(b, 1)
                    phase15(b)
                    phase2(b)

        if cfg['repeat'] > 1:
            for rep in range(cfg['repeat']):
                emit_body(rep)
        else:
            emit_body(0)

    nc.compile()
    return nc


# ---------------------------------------------------------------------------
# host-side weight prep
# ---------------------------------------------------------------------------

def prep_weights(inputs, cfg):
    import ml_dtypes
    conv_np = ml_dtypes.bfloat16 if cfg['conv_bf16'] else np.float32
    f32 = np.float32
    bf = ml_dtypes.bfloat16 if cfg.get('gemm_bf16', True) else np.float32
    qkv_w = np.asarray(inputs['qkv_w'], f32)
    wq_t = np.ascontiguousarray(qkv_w[0:DIM].T).astype(bf)
    wk_t = np.ascontiguousarray(qkv_w[DIM:2 * DIM].T).astype(bf)
    wv_t = np.ascontiguousarray(qkv_w[2 * DIM:3 * DIM].T).astype(bf)
    proj_t = np.ascontiguousarray(np.asarray(inputs['proj_w'], f32).T).astype(bf)

    def diag_taps(wconv):
        w = np.asarray(wconv, f32).reshape(DIM, 9)
        out = np.zeros((NH, 9, HD, HD), f32)
        idx = np.arange(HD)
        for c in range(NH):
            for j in range(9):
                out[c, j, idx, idx] = w[c * HD:(c + 1) * HD, j]
        return out.astype(conv_np)

    def wvecs(wconv):
        w = np.asarray(wconv, f32).reshape(DIM, 9)
        return np.ascontiguousarray(w.reshape(NH, HD, 9))

    npix = cfg['H'] * W
    return dict(
        wq_t=wq_t, wk_t=wk_t, wv_t=wv_t, proj_t=proj_t,
        diag1q=diag_taps(inputs['sq_w']),
        diag1k=diag_taps(inputs['sk_w']),
        diag2=diag_taps(inputs['dwc_w']),
        wvec1q=wvecs(inputs['sq_w']),
        wvec1k=wvecs(inputs['sk_w']),
        sq_b=np.asarray(inputs['sq_b'], f32).reshape(DIM, 1),
        sk_b=np.asarray(inputs['sk_b'], f32).reshape(DIM, 1),
        dwc_b=np.asarray(inputs['dwc_b'], f32).reshape(DIM, 1),
        proj_b=np.asarray(inputs['proj_b'], f32).reshape(DIM, 1),
        se_w1q=np.ascontiguousarray(
            np.asarray(inputs['cq_w1'], f32).transpose(0, 2, 1) / npix),
        se_b1q=np.asarray(inputs['cq_b1'], f32).reshape(NH, HD4, 1),
        se_w2q=np.ascontiguousarray(
            np.asarray(inputs['cq_w2'], f32).transpose(0, 2, 1)),
        se_b2q=np.asarray(inputs['cq_b2'], f32).reshape(NH, HD, 1),
        se_w1k=np.ascontiguousarray(
            np.asarray(inputs['ck_w1'], f32).transpose(0, 2, 1) / npix),
        se_b1k=np.asarray(inputs['ck_b1'], f32).reshape(NH, HD4, 1),
        se_w2k=np.ascontiguousarray(
            np.asarray(inputs['ck_w2'], f32).transpose(0, 2, 1)),
        se_b2k=np.asarray(inputs['ck_b2'], f32).reshape(NH, HD, 1),
    )


_CACHE = {}


def _get_compiled(cfg_key, cfg):
    if cfg_key not in _CACHE:
        _CACHE[cfg_key] = build_nc(cfg)
    return _CACHE[cfg_key]


def kernel(**inputs):
    import ml_dtypes
    from concourse import bass_utils
    cfg = default_cfg()
    nc = _get_compiled('main', cfg)
    w = prep_weights(inputs, cfg)
    x32 = np.asarray(inputs['x'], np.float32)
    x = x32.astype(ml_dtypes.bfloat16) if cfg['gemm_bf16'] else x32
    BL = cfg['b_local']
    in_maps = []
    for core in range(N_CORES):
        m = dict(w)
        m['x'] = np.ascontiguousarray(x[core * BL:(core + 1) * BL])
        in_maps.append(m)
    res = bass_utils.run_bass_kernel_spmd(nc, in_maps, core_ids=list(range(N_CORES)))
    out = np.empty((B, DIM, H_FULL, W), np.float32)
    for core in range(N_CORES):
        out[core * BL:(core + 1) * BL] = res.results[core]['out']
    return out



# revision 15
# speedup vs baseline: 1.0547x; 1.0547x over previous
"""Trainium2 Bass kernel for nn_CASAtt_MultiHead_v1 (CAS attention block).

Reference computation (per sample):
    qkv = 1x1 conv (qkv_w) -> q, k, v                        [512, 56, 56] each
    q <- SE(dwconv3x3(q, sq_w, sq_b))   (per-head squeeze-excite)
    k <- SE(dwconv3x3(k, sk_w, sk_b))
    out = proj(dwconv3x3(q + k, dwc_w, dwc_b) * v) + proj_b + x

Distribution: data-parallel over batch, 2 samples per NeuronCore x 8 cores.

v3 design (measured on HW via micro-benchmarks):
* qkv + proj GEMMs in fp8(e4m3) with MatmulPerfMode.DoubleRow
  (553ns per K=512,N=448 output block vs ~750ns bf16).  Full-chain fp8
  emulation on host: rel err ~6e-3 << 2e-2 gate.
* Depthwise convs as diag-matrix matmuls on the PE, packed as 64x64
  tile_position blocks: the two diagonal 64-blocks of a chunk pair
  (oc even/odd) map to 4 distinct array positions by giving the odd
  chunk a half-swap rotation -> 4 concurrent moving streams, measured
  607 Ge/s vs 256 Ge/s for plain 128x128 diag matmuls.  The odd chunks'
  m image ends up half-swapped; all consumers (conv2 weights, v/proj
  GEMM blocks, biases) are host-permuted to match, nothing on-chip
  un-rotates.
* conv2 runs per-chunk on a configurable engine: 'pe' (pair-rotated,
  output back to natural layout), 'dve' (tensor_scalar@4x +
  tensor_tensor@2x chain, ~143 Ge/s), or 'dva' (ACT scale-copies +
  GpSimd tensor_tensor accumulate) to balance engine load.
* SE pooling approximated: mean(dwconv(q)) ~= (sum_taps w)*mean(q)
  (border terms shift s by ~1e-5 of 0.5); mean(q) comes free from the
  accum_out of the q/k GEMM drains, so conv1 outputs never materialize:
  m = dw3_q(q)*s_q + dw3_k(k)*s_k accumulates all 18 taps of both
  branches into one PSUM group with s folded into the diag weights.
* o2 = (conv2+b)*v is built by a single STT/TT per tile directly into a
  [128, 4, NPIX] fp8 tile (DoubleRow moving operand for proj).
* Residual + output in bf16 (abs budget 0.109 at absmax 5.45; bf16
  costs ~0.011); host converts the bf16 output back to f32.
* Mixed-dtype tensor-tensor DVE ops (psum f32 + bf16) NaN on HW; all
  tensor-tensor ops keep operand dtypes equal.
"""

import numpy as np

DIM = 512
NH = 4
HD = 128
HD4 = 32
B, H, W = 16, 56, 56
N_CORES = 8
BL = B // N_CORES

TH = 8                  # rows per tile
NT = H // TH            # 7
TN = TH * W             # 448
WP = W + 2              # 58 padded row stride
TPAD = TH * WP          # 464
PADN = (H + 2) * WP     # 3364
NPIX = H * W            # 3136

TAPS = [(dy, dx) for dy in (-1, 0, 1) for dx in (-1, 0, 1)]
NTAP = 9


def default_cfg():
    return dict(
        qkv_fp8=1,
        proj_fp8=1,
        # per-chunk conv2 engine: 'pe' chunks must come in (even, odd)
        # pairs sharing the same engine
        conv2_assign='pe,pe,dve,dva',
        mconv_G=2,
        conv2_G=2,
    )


# layout helpers ------------------------------------------------------------

def _lay_m(oc):
    """channel-within-chunk at partition p of m[oc] (PE 64-block rot)."""
    p = np.arange(HD)
    return 64 * ((p // 64 - oc) % 2) + p % 64


def _lay_id(oc):
    return np.arange(HD)


def layouts(cfg):
    c2a = cfg['conv2_assign'].split(',')
    lay_m = [_lay_m(oc) for oc in range(NH)]          # m buffer layout
    lay_c2 = []                                       # o2 / v layout
    for oc in range(NH):
        if c2a[oc] == 'pe':
            lay_c2.append(_lay_id(oc))                # pair rotation undoes
        else:
            lay_c2.append(lay_m[oc])                  # per-partition engines
    return c2a, lay_m, lay_c2


def build_nc(cfg):
    import concourse.bass as bass
    import concourse.mybir as mybir
    import concourse.tile as tile
    from concourse import bacc
    from contextlib import ExitStack

    f32 = mybir.dt.float32
    bf16 = mybir.dt.bfloat16
    fp8 = mybir.dt.float8e4
    AF = mybir.ActivationFunctionType
    AL = mybir.AluOpType
    DR = mybir.MatmulPerfMode.DoubleRow

    xdt = fp8 if cfg['qkv_fp8'] else bf16
    odt = fp8 if cfg['proj_fp8'] else bf16
    c2a, _, _ = layouts(cfg)
    MG, CG = cfg['mconv_G'], cfg['conv2_G']

    nc = bacc.Bacc("TRN2", target_bir_lowering=False, debug=False,
                   enable_asserts=False, num_devices=N_CORES)

    # ---------------- DRAM I/O ----------------
    x8_d = nc.dram_tensor("x8", [BL, NH, HD, H, W], xdt,
                          kind="ExternalInput").ap()
    xb_d = nc.dram_tensor("xb", [BL, NH, HD, H, W], bf16,
                          kind="ExternalInput").ap()
    out_d = nc.dram_tensor("out", [BL, NH, HD, H, W], bf16,
                           kind="ExternalOutput").ap()
    wq_d = nc.dram_tensor("wq", [NH, HD, NH, HD], xdt, kind="ExternalInput").ap()
    wk_d = nc.dram_tensor("wk", [NH, HD, NH, HD], xdt, kind="ExternalInput").ap()
    wv_d = nc.dram_tensor("wv", [NH, HD, NH, HD], xdt, kind="ExternalInput").ap()
    wp_d = nc.dram_tensor("wp", [NH, HD, NH, HD], odt, kind="ExternalInput").ap()
    # 64-wide diag blocks for PE convs
    dgq_d = nc.dram_tensor("dgq", [NH, HD, NTAP, 64], bf16,
                           kind="ExternalInput").ap()
    dgk_d = nc.dram_tensor("dgk", [NH, HD, NTAP, 64], bf16,
                           kind="ExternalInput").ap()
    dg2_d = nc.dram_tensor("dg2", [NH, HD, NTAP, 64], bf16,
                           kind="ExternalInput").ap()
    # per-partition conv2 weights for dve/dva chunks
    wv2_d = nc.dram_tensor("wvec2", [NH, HD, NTAP], f32,
                           kind="ExternalInput").ap()
    sew1_d = [nc.dram_tensor(n, [NH, HD, HD4], f32, kind="ExternalInput").ap()
              for n in ("sew1q", "sew1k")]
    seb1_d = [nc.dram_tensor(n, [NH, HD4, 1], f32, kind="ExternalInput").ap()
              for n in ("seb1q", "seb1k")]
    sew2n_d = [nc.dram_tensor(n, [NH, HD4, HD], f32, kind="ExternalInput").ap()
               for n in ("sew2qn", "sew2kn")]
    seb2n_d = [nc.dram_tensor(n, [NH, HD, 1], f32, kind="ExternalInput").ap()
               for n in ("seb2qn", "seb2kn")]
    sew2r_d = [nc.dram_tensor(n, [NH, HD4, HD], f32, kind="ExternalInput").ap()
               for n in ("sew2qr", "sew2kr")]
    seb2r_d = [nc.dram_tensor(n, [NH, HD, 1], f32, kind="ExternalInput").ap()
               for n in ("seb2qr", "seb2kr")]
    bqr_d = nc.dram_tensor("bq_rot", [DIM, 1], f32, kind="ExternalInput").ap()
    bkr_d = nc.dram_tensor("bk_rot", [DIM, 1], f32, kind="ExternalInput").ap()
    dwcb_d = nc.dram_tensor("dwcb_l", [DIM, 1], f32, kind="ExternalInput").ap()
    projb_d = nc.dram_tensor("projb", [DIM, 1], f32, kind="ExternalInput").ap()

    with tile.TileContext(nc) as tc, ExitStack() as ctx:
        const = ctx.enter_context(tc.tile_pool(name="const", bufs=1))
        big = ctx.enter_context(tc.tile_pool(name="big", bufs=1))
        st = ctx.enter_context(tc.tile_pool(name="st", bufs=2))
        pp = ctx.enter_context(tc.tile_pool(name="pp", bufs=8, space="PSUM"))

        qpad = [big.tile([HD, PADN + 2], bf16, name=f"qpad{c}") for c in range(NH)]
        kpad = [big.tile([HD, PADN + 2], bf16, name=f"kpad{c}") for c in range(NH)]
        mpad = [big.tile([HD, PADN + 2], bf16, name=f"mpad{c}") for c in range(NH)]
        vsb = [big.tile([HD, NPIX], bf16, name=f"vsb{c}") for c in range(NH)]
        o2all = big.tile([HD, NH, NPIX], odt, name="o2all")
        acc = [big.tile([HD, H * WP], bf16, name=f"acc{i}") for i in range(3)]

        def pad3(t):
            return t[:, 1:1 + PADN].rearrange("p (h w) -> p h w", w=WP)

        # zero pad cells: top row(+slop), bottom row(+slop), col pads
        for t in qpad + kpad + mpad:
            nc.vector.memset(t[:, 0:1 + WP + 1], 0.0)
            nc.vector.memset(t[:, 1 + (H + 1) * WP:PADN + 2], 0.0)
            nc.vector.memset(
                t[:, 1 + WP + W + 1:1 + WP + W + 1 + H * WP].rearrange(
                    "p (h w) -> p h w", w=WP)[:, :, 0:2], 0.0)

        # ---------- weights ----------
        gemm_w = {}
        for nm, wd in (('q', wq_d), ('k', wk_d), ('v', wv_d)):
            for oc in range(NH):
                t = const.tile([HD, NH, HD], xdt, name=f"w{nm}{oc}")
                nc.sync.dma_start(t, wd[oc])
                gemm_w[nm, oc] = t
        for oc in range(NH):
            t = const.tile([HD, NH, HD], odt, name=f"wp{oc}")
            nc.sync.dma_start(t, wp_d[oc])
            gemm_w['p', oc] = t
        dg = {}
        for key, wd in (('q', dgq_d), ('k', dgk_d), ('2', dg2_d)):
            for oc in range(NH):
                if key == '2' and c2a[oc] != 'pe':
                    continue
                t = const.tile([HD, NTAP, 64], bf16, name=f"dg{key}{oc}")
                nc.sync.dma_start(t, wd[oc])
                dg[key, oc] = t
        dgs = {}
        for key in ('q', 'k'):
            for oc in range(NH):
                dgs[key, oc] = const.tile([HD, NTAP, 64], bf16,
                                          name=f"dgs{key}{oc}")
        wvec2 = {}
        for oc in range(NH):
            if c2a[oc] != 'pe':
                t = const.tile([HD, NTAP], f32, name=f"wvec2_{oc}")
                nc.sync.dma_start(t, wv2_d[oc])
                wvec2[oc] = t

        se = {}
        for nm, drams in (('w1', sew1_d), ('b1', seb1_d), ('w2n', sew2n_d),
                          ('b2n', seb2n_d), ('w2r', sew2r_d), ('b2r', seb2r_d)):
            shp = {'w1': [HD, HD4], 'b1': [HD4, 1], 'w2n': [HD4, HD],
                   'b2n': [HD, 1], 'w2r': [HD4, HD], 'b2r': [HD, 1]}[nm]
            for br in range(2):
                for oc in range(NH):
                    t = const.tile(shp, f32, name=f"se_{nm}_{br}_{oc}")
                    nc.sync.dma_start(t, drams[br][oc])
                    se[nm, br, oc] = t
        bqr, bkr, dwcb, projb = [], [], [], []
        for oc in range(NH):
            sl = slice(oc * HD, (oc + 1) * HD)
            for lst, dram, nm in ((bqr, bqr_d, 'bqr'), (bkr, bkr_d, 'bkr'),
                                  (dwcb, dwcb_d, 'dwcb'),
                                  (projb, projb_d, 'pjb')):
                t = const.tile([HD, 1], f32, name=f"{nm}{oc}")
                nc.sync.dma_start(t, dram[sl])
                lst.append(t)

        def tap_sl(buf, t, j):
            dy, dx = TAPS[j]
            base = 1 + (t * TH + 1) * WP + dy * WP + dx
            return buf[:, base:base + TPAD]

        def gemm(ps, wtile, xtile, tsl, dr):
            if dr:
                for ks in range(0, NH, 2):
                    nc.tensor.matmul(ps, wtile[:, ks:ks + 2, :],
                                     xtile[:, ks:ks + 2, tsl],
                                     start=(ks == 0), stop=(ks == 2),
                                     perf_mode=DR)
            else:
                for kc in range(NH):
                    nc.tensor.matmul(ps, wtile[:, kc, :], xtile[:, kc, tsl],
                                     start=(kc == 0), stop=(kc == NH - 1))

        # ================= per-sample body =================
        def emit_sample(b):
            stats = st.tile([HD, 2 * NH * NT], f32, name=f"stats{b}",
                            tag="stats")
            # ---- PH_A: q,k GEMM + fused pooling ----
            for t in range(NT):
                r0 = t * TH
                xt = st.tile([HD, NH, TN], xdt, tag="xt", bufs=3,
                             name=f"xt{b}_{t}")
                nc.sync.dma_start(
                    xt.rearrange("p c (h w) -> p c h w", w=W),
                    x8_d[b, :, :, r0:r0 + TH, :].rearrange(
                        "c p h w -> p c h w"))
                for br, (nm, dst) in enumerate((('q', qpad), ('k', kpad))):
                    for oc in range(NH):
                        ps = pp.tile([HD, TN], f32, tag="ps",
                                     name=f"g{b}_{br}_{t}_{oc}")
                        gemm(ps, gemm_w[nm, oc], xt, slice(None),
                             cfg['qkv_fp8'])
                        nc.scalar.activation(
                            pad3(dst[oc])[:, 1 + r0:1 + r0 + TH, 1:1 + W],
                            ps.rearrange("p (h w) -> p h w", w=W),
                            AF.Identity,
                            accum_out=stats[:, (br * NH + oc) * NT + t:
                                            (br * NH + oc) * NT + t + 1])
            # ---- V GEMM (covers SE latency) ----
            for t in range(NT):
                r0 = t * TH
                xt = st.tile([HD, NH, TN], xdt, tag="xt", bufs=3,
                             name=f"xtv{b}_{t}")
                nc.sync.dma_start(
                    xt.rearrange("p c (h w) -> p c h w", w=W),
                    x8_d[b, :, :, r0:r0 + TH, :].rearrange(
                        "c p h w -> p c h w"))
                for oc in range(NH):
                    ps = pp.tile([HD, TN], f32, tag="ps", name=f"v{b}_{t}_{oc}")
                    gemm(ps, gemm_w['v', oc], xt, slice(None), cfg['qkv_fp8'])
                    nc.scalar.copy(vsb[oc][:, t * TN:(t + 1) * TN], ps)
            # ---- SE ----
            s_rot = [[None] * NH for _ in range(2)]
            for br in range(2):
                for oc in range(NH):
                    pooled = const.tile([HD, 1], f32, tag="pooled", bufs=4,
                                        name=f"pool{b}_{br}_{oc}")
                    i0 = (br * NH + oc) * NT
                    nc.vector.tensor_reduce(pooled, stats[:, i0:i0 + NT],
                                            mybir.AxisListType.X, AL.add)
                    ps1 = pp.tile([HD4, 1], f32, tag="ps",
                                  name=f"se1_{b}_{br}_{oc}")
                    nc.tensor.matmul(ps1, se['w1', br, oc], pooled,
                                     start=True, stop=True)
                    hvec = const.tile([HD4, 1], f32, tag="hvec", bufs=4,
                                      name=f"h{b}_{br}_{oc}")
                    nc.scalar.activation(hvec, ps1, AF.Relu,
                                         bias=se['b1', br, oc])
                    ps2 = pp.tile([HD, 1], f32, tag="ps",
                                  name=f"se2_{b}_{br}_{oc}")
                    nc.tensor.matmul(ps2, se['w2n', br, oc], hvec,
                                     start=True, stop=True)
                    s_nat = const.tile([HD, 1], f32, tag="s_nat", bufs=8,
                                       name=f"sn{b}_{br}_{oc}")
                    nc.scalar.activation(s_nat, ps2, AF.Sigmoid,
                                         bias=se['b2n', br, oc])
                    ps2r = pp.tile([HD, 1], f32, tag="ps",
                                   name=f"se2r_{b}_{br}_{oc}")
                    nc.tensor.matmul(ps2r, se['w2r', br, oc], hvec,
                                     start=True, stop=True)
                    sr = const.tile([HD, 1], f32, tag="s_rot", bufs=8,
                                    name=f"sr{b}_{br}_{oc}")
                    nc.scalar.activation(sr, ps2r, AF.Sigmoid,
                                         bias=se['b2r', br, oc])
                    s_rot[br][oc] = sr
                    key = 'q' if br == 0 else 'k'
                    nc.vector.tensor_scalar(dgs[key, oc], dg[key, oc], s_nat,
                                            None, AL.mult)
            bias_m = []
            for oc in range(NH):
                tmp = const.tile([HD, 1], f32, tag="bm_tmp", bufs=2,
                                 name=f"bmt{b}_{oc}")
                nc.vector.tensor_scalar(tmp, bqr[oc], s_rot[0][oc],
                                        None, AL.mult)
                bm = const.tile([HD, 1], f32, tag="bias_m", bufs=4,
                                name=f"bm{b}_{oc}")
                nc.vector.scalar_tensor_tensor(bm, bkr[oc], s_rot[1][oc], tmp,
                                               AL.mult, AL.add)
                bias_m.append(bm)

            # ---- m-conv on PE: 18 taps, 64x64 blocks, pair rotation ----
            for g in range(0, NT, MG):
                gts = list(range(g, min(g + MG, NT)))
                pst = [[pp.tile([HD, TPAD], f32, tag="ps",
                                name=f"m{b}_{t}_{oc}") for oc in range(NH)]
                       for t in gts]
                for jj in range(2 * NTAP):
                    br, j = divmod(jj, NTAP)
                    key, src = ('q', qpad) if br == 0 else ('k', kpad)
                    for pair in range(2):
                        for cc in range(2):
                            oc = 2 * pair + cc
                            for a in range(2):
                                bb = (a + cc) % 2
                                for ti in range(len(gts)):
                                    nc.tensor.matmul(
                                        pst[ti][oc][64 * bb:64 * bb + 64, :],
                                        dgs[key, oc][64 * a:64 * a + 64, j, :],
                                        tap_sl(src[oc], gts[ti], j)[
                                            64 * a:64 * a + 64, :],
                                        start=(jj == 0),
                                        stop=(jj == 2 * NTAP - 1),
                                        tile_position=(64 * a, 64 * bb),
                                        skip_group_check=True)
                for ti, t in enumerate(gts):
                    for oc in range(NH):
                        nc.scalar.activation(
                            pad3(mpad[oc])[:, 1 + t * TH:1 + t * TH + TH,
                                           1:1 + W],
                            pst[ti][oc].rearrange(
                                "p (h w) -> p h w", w=WP)[:, :, 1:1 + W],
                            AF.Identity, bias=bias_m[oc])

            # ---- conv2 per chunk on assigned engine, then o2 = (c2+b)*v ----
            pe_ocs = [oc for oc in range(NH) if c2a[oc] == 'pe']
            for g in range(0, NT, CG):
                gts = list(range(g, min(g + CG, NT)))
                if pe_ocs:
                    pst = [{oc: pp.tile([HD, TPAD], f32, tag="ps",
                                        name=f"c2{b}_{t}_{oc}")
                            for oc in pe_ocs} for t in gts]
                    for j in range(NTAP):
                        for oc in pe_ocs:
                            cc = oc % 2
                            for bb in range(2):
                                b2 = (bb + cc) % 2
                                for ti in range(len(gts)):
                                    nc.tensor.matmul(
                                        pst[ti][oc][64 * b2:64 * b2 + 64, :],
                                        dg['2', oc][64 * bb:64 * bb + 64, j, :],
                                        tap_sl(mpad[oc], gts[ti], j)[
                                            64 * bb:64 * bb + 64, :],
                                        start=(j == 0), stop=(j == NTAP - 1),
                                        tile_position=(64 * bb, 64 * b2),
                                        skip_group_check=True)
                    for ti, t in enumerate(gts):
                        for oc in pe_ocs:
                            c2t = st.tile([HD, TN], bf16, tag="c2t", bufs=4,
                                          name=f"c2t{b}_{t}_{oc}")
                            nc.scalar.activation(
                                c2t.rearrange("p (h w) -> p h w", w=W),
                                pst[ti][oc].rearrange(
                                    "p (h w) -> p h w", w=WP)[:, :, 1:1 + W],
                                AF.Identity, bias=dwcb[oc])
                            nc.vector.tensor_tensor(
                                o2all[:, oc, t * TN:(t + 1) * TN], c2t,
                                vsb[oc][:, t * TN:(t + 1) * TN], AL.mult)
            # dve / dva chunks: whole-image tap chains in SBUF
            for oc in range(NH):
                eng = c2a[oc]
                if eng == 'pe':
                    continue
                wv2 = wvec2[oc]
                cur, nxt, tmp = 0, 1, 2
                dy, dx = TAPS[0]
                base = 1 + WP + dy * WP + dx      # rows 1..H, all cols
                cnt = H * WP
                nc.vector.tensor_scalar(
                    acc[cur][:, 0:cnt],
                    mpad[oc][:, base:base + cnt], wv2[:, 0:1], None, AL.mult)
                for j in range(1, NTAP):
                    dy, dx = TAPS[j]
                    base = 1 + WP + dy * WP + dx
                    msl = mpad[oc][:, base:base + cnt]
                    if eng == 'dve':
                        nc.vector.tensor_scalar(
                            acc[tmp][:, 0:cnt], msl, wv2[:, j:j + 1],
                            None, AL.mult)
                        nc.vector.tensor_tensor(
                            acc[nxt][:, 0:cnt], acc[cur][:, 0:cnt],
                            acc[tmp][:, 0:cnt], AL.add)
                    else:  # dva: ACT scale-copy + Pool accumulate
                        nc.scalar.activation(
                            acc[tmp][:, 0:cnt], msl, AF.Copy,
                            scale=wv2[:, j:j + 1])
                        nc.gpsimd.tensor_tensor(
                            acc[nxt][:, 0:cnt], acc[cur][:, 0:cnt],
                            acc[tmp][:, 0:cnt], AL.add)
                    cur, nxt, tmp = nxt, tmp, cur
                # o2 = (acc + dwc_b) * v  -- acc rows 1..H, interior cols
                acc3 = acc[cur][:, 0:cnt].rearrange("p (h w) -> p h w", w=WP)
                nc.vector.scalar_tensor_tensor(
                    o2all[:, oc, :].rearrange("p (h w) -> p h w", w=W),
                    acc3[:, :, 1:1 + W], dwcb[oc],
                    vsb[oc].rearrange("p (h w) -> p h w", w=W),
                    AL.add, AL.mult)

            # ---- proj GEMM + residual + out ----
            for t in range(NT):
                r0 = t * TH
                xb = st.tile([HD, NH, TN], bf16, tag="xb", bufs=2,
                             name=f"xb{b}_{t}")
                nc.sync.dma_start(
                    xb.rearrange("p c (h w) -> p c h w", w=W),
                    xb_d[b, :, :, r0:r0 + TH, :].rearrange(
                        "c p h w -> p c h w"))
                ot = st.tile([HD, NH, TN], bf16, tag="ot", bufs=2,
                             name=f"ot{b}_{t}")
                for oc in range(NH):
                    ps = pp.tile([HD, TN], f32, tag="ps", name=f"p{b}_{t}_{oc}")
                    gemm(ps, gemm_w['p', oc],
                         o2all.rearrange("p c n -> p c n"),
                         slice(t * TN, (t + 1) * TN), cfg['proj_fp8'])
                    pt = st.tile([HD, TN], bf16, tag="pt", bufs=4,
                                 name=f"pt{b}_{t}_{oc}")
                    nc.scalar.activation(pt, ps, AF.Identity, bias=projb[oc])
                    nc.vector.tensor_tensor(ot[:, oc, :], pt, xb[:, oc, :],
                                            AL.add)
                nc.sync.dma_start(
                    out_d[b, :, :, r0:r0 + TH, :].rearrange(
                        "c p h w -> p c h w"),
                    ot.rearrange("p c (h w) -> p c h w", w=W))

        for b in range(BL):
            emit_sample(b)

    nc.compile()
    return nc


# ---------------------------------------------------------------------------
# host-side weight prep
# ---------------------------------------------------------------------------

def prep_weights(inputs, cfg):
    import ml_dtypes
    f32 = np.float32
    bf = ml_dtypes.bfloat16
    e4 = ml_dtypes.float8_e4m3
    xdt = e4 if cfg['qkv_fp8'] else bf
    odt = e4 if cfg['proj_fp8'] else bf
    c2a, lay_m, lay_c2 = layouts(cfg)

    qkv_w = np.asarray(inputs['qkv_w'], f32)
    proj_w = np.asarray(inputs['proj_w'], f32)

    def gemm_blocks(wmat, row_perm, dt):
        # lhsT blocks [oc][k_part, k_sub, m]; row_perm permutes output chans
        out = np.empty((NH, HD, NH, HD), f32)
        for oc in range(NH):
            rows = wmat[oc * HD:(oc + 1) * HD]
            if row_perm is not None:
                rows = rows[row_perm[oc]]
            out[oc] = rows.reshape(HD, NH, HD).transpose(2, 1, 0)
        return np.ascontiguousarray(out).astype(dt)

    def proj_blocks(wmat):
        # input (o2) channels are in lay_c2 layout: permute columns
        out = np.empty((NH, HD, NH, HD), f32)
        for oc in range(NH):
            cols = wmat[oc * HD:(oc + 1) * HD].reshape(HD, NH, HD)
            perm_cols = np.empty_like(cols)
            for kc in range(NH):
                perm_cols[:, kc, :] = cols[:, kc, lay_c2[kc]]
            out[oc] = perm_cols.transpose(2, 1, 0)
        return np.ascontiguousarray(out).astype(odt)

    def diag_blocks(wconv, row_perm=None):
        w = np.asarray(wconv, f32).reshape(DIM, NTAP)
        out = np.zeros((NH, HD, NTAP, 64), f32)
        for oc in range(NH):
            ch = np.arange(HD) if row_perm is None else row_perm[oc]
            for p in range(HD):
                out[oc, p, :, p % 64] = w[oc * HD + ch[p]]
        return out.astype(bf)

    def wvec_l(wconv):
        # per-partition conv2 weights in the m layout
        w = np.asarray(wconv, f32).reshape(DIM, NTAP)
        out = np.empty((NH, HD, NTAP), f32)
        for oc in range(NH):
            out[oc] = w[oc * HD + lay_m[oc]]
        return out

    npix = float(NPIX)
    wsum_q = np.asarray(inputs['sq_w'], f32).reshape(DIM, NTAP).sum(1)
    wsum_k = np.asarray(inputs['sk_w'], f32).reshape(DIM, NTAP).sum(1)

    def se_prep(w1, b1, w2, b2, wsum, cbias):
        w1 = np.asarray(w1, f32)
        b1 = np.asarray(b1, f32)
        w2 = np.asarray(w2, f32)
        b2 = np.asarray(b2, f32)
        cbias = np.asarray(cbias, f32).reshape(NH, HD)
        sew1 = np.empty((NH, HD, HD4), f32)
        seb1 = np.empty((NH, HD4, 1), f32)
        sew2n = np.empty((NH, HD4, HD), f32)
        seb2n = np.asarray(b2, f32).reshape(NH, HD, 1).copy()
        sew2r = np.empty((NH, HD4, HD), f32)
        seb2r = np.empty((NH, HD, 1), f32)
        for oc in range(NH):
            sew1[oc] = (w1[oc] * (wsum[oc * HD:(oc + 1) * HD] / npix)[None, :]).T
            seb1[oc] = (b1[oc] + w1[oc] @ cbias[oc]).reshape(HD4, 1)
            sew2n[oc] = w2[oc].T
            r = lay_m[oc]
            sew2r[oc] = w2[oc][r].T
            seb2r[oc] = b2[oc][r].reshape(HD, 1)
        return dict(w1=sew1, b1=seb1, w2n=sew2n, b2n=seb2n, w2r=sew2r,
                    b2r=seb2r)

    sq_b = np.asarray(inputs['sq_b'], f32)
    sk_b = np.asarray(inputs['sk_b'], f32)
    dwc_b = np.asarray(inputs['dwc_b'], f32)
    se_q = se_prep(inputs['cq_w1'], inputs['cq_b1'], inputs['cq_w2'],
                   inputs['cq_b2'], wsum_q, sq_b)
    se_k = se_prep(inputs['ck_w1'], inputs['ck_b1'], inputs['ck_w2'],
                   inputs['ck_b2'], wsum_k, sk_b)

    bq_rot = np.empty((DIM, 1), f32)
    bk_rot = np.empty((DIM, 1), f32)
    dwcb_l = np.empty((DIM, 1), f32)
    for oc in range(NH):
        bq_rot[oc * HD:(oc + 1) * HD, 0] = sq_b[oc * HD + lay_m[oc]]
        bk_rot[oc * HD:(oc + 1) * HD, 0] = sk_b[oc * HD + lay_m[oc]]
        # conv2 'pe' chunks add dwc_b at the (natural-layout) drain;
        # dve/dva chunks add it inside the STT in m layout
        perm = np.arange(HD) if c2a[oc] == 'pe' else lay_m[oc]
        dwcb_l[oc * HD:(oc + 1) * HD, 0] = dwc_b[oc * HD + perm]

    w = dict(
        wq=gemm_blocks(qkv_w[0:DIM], None, xdt),
        wk=gemm_blocks(qkv_w[DIM:2 * DIM], None, xdt),
        wv=gemm_blocks(qkv_w[2 * DIM:3 * DIM], lay_c2, xdt),
        wp=proj_blocks(proj_w),
        dgq=diag_blocks(inputs['sq_w']),
        dgk=diag_blocks(inputs['sk_w']),
        dg2=diag_blocks(inputs['dwc_w'], row_perm=lay_m),
        wvec2=wvec_l(inputs['dwc_w']),
        bq_rot=bq_rot, bk_rot=bk_rot, dwcb_l=dwcb_l,
        projb=np.asarray(inputs['proj_b'], f32).reshape(DIM, 1),
    )
    for nm, sed in (('q', se_q), ('k', se_k)):
        w[f'sew1{nm}'] = sed['w1']
        w[f'seb1{nm}'] = sed['b1']
        w[f'sew2{nm}n'] = sed['w2n']
        w[f'seb2{nm}n'] = sed['b2n']
        w[f'sew2{nm}r'] = sed['w2r']
        w[f'seb2{nm}r'] = sed['b2r']
    return w


_CACHE = {}


def _get_compiled(cfg_key, cfg):
    if cfg_key not in _CACHE:
        _CACHE[cfg_key] = build_nc(cfg)
    return _CACHE[cfg_key]


def make_in_maps(inputs, cfg):
    import ml_dtypes
    w = prep_weights(inputs, cfg)
    x32 = np.asarray(inputs['x'], np.float32).reshape(B, NH, HD, H, W)
    xdt = ml_dtypes.float8_e4m3 if cfg['qkv_fp8'] else ml_dtypes.bfloat16
    x8 = np.ascontiguousarray(x32).astype(xdt)
    xbf = np.ascontiguousarray(x32).astype(ml_dtypes.bfloat16)
    in_maps = []
    for core in range(N_CORES):
        m = dict(w)
        m['x8'] = x8[core * BL:(core + 1) * BL]
        m['xb'] = xbf[core * BL:(core + 1) * BL]
        in_maps.append(m)
    return in_maps


def gather_out(results):
    out = np.empty((B, DIM, H, W), np.float32)
    for core in range(N_CORES):
        out[core * BL:(core + 1) * BL] = np.asarray(
            results[core]['out'], np.float32).reshape(BL, DIM, H, W)
    return out


def kernel(**inputs):
    from concourse import bass_utils
    cfg = default_cfg()
    nc = _get_compiled('main', cfg)
    in_maps = make_in_maps(inputs, cfg)
    res = bass_utils.run_bass_kernel_spmd(nc, in_maps,
                                          core_ids=list(range(N_CORES)))
    return gather_out(res.results)


# revision 18
# speedup vs baseline: 1.5859x; 1.5037x over previous
"""Trainium2 Bass kernel for nn_CASAtt_MultiHead_v1 (CAS attention block).

Reference computation (per sample):
    qkv = 1x1 conv (qkv_w) -> q, k, v                        [512, 56, 56] each
    q <- SE(dwconv3x3(q, sq_w, sq_b))   (per-head squeeze-excite)
    k <- SE(dwconv3x3(k, sk_w, sk_b))
    out = proj(dwconv3x3(q + k, dwc_w, dwc_b) * v) + proj_b + x

Distribution: data-parallel over batch, 2 samples per NeuronCore x 8 cores.

v3 design (measured on HW via micro-benchmarks):
* qkv + proj GEMMs in fp8(e4m3) with MatmulPerfMode.DoubleRow
  (553ns per K=512,N=448 output block vs ~750ns bf16).  Full-chain fp8
  emulation on host: rel err ~6e-3 << 2e-2 gate.
* Depthwise convs as diag-matrix matmuls on the PE, packed as 64x64
  tile_position blocks: the two diagonal 64-blocks of a chunk pair
  (oc even/odd) map to 4 distinct array positions by giving the odd
  chunk a half-swap rotation -> 4 concurrent moving streams, measured
  607 Ge/s vs 256 Ge/s for plain 128x128 diag matmuls.  The odd chunks'
  m image ends up half-swapped; all consumers (conv2 weights, v/proj
  GEMM blocks, biases) are host-permuted to match, nothing on-chip
  un-rotates.
* conv2 runs per-chunk on a configurable engine: 'pe' (pair-rotated,
  output back to natural layout), 'dve' (tensor_scalar@4x +
  tensor_tensor@2x chain, ~143 Ge/s), or 'dva' (ACT scale-copies +
  GpSimd tensor_tensor accumulate) to balance engine load.
* SE pooling approximated: mean(dwconv(q)) ~= (sum_taps w)*mean(q)
  (border terms shift s by ~1e-5 of 0.5); mean(q) comes free from the
  accum_out of the q/k GEMM drains, so conv1 outputs never materialize:
  m = dw3_q(q)*s_q + dw3_k(k)*s_k accumulates all 18 taps of both
  branches into one PSUM group with s folded into the diag weights.
* o2 = (conv2+b)*v is built by a single STT/TT per tile directly into a
  [128, 4, NPIX] fp8 tile (DoubleRow moving operand for proj).
* Residual + output in bf16 (abs budget 0.109 at absmax 5.45; bf16
  costs ~0.011); host converts the bf16 output back to f32.
* Mixed-dtype tensor-tensor DVE ops (psum f32 + bf16) NaN on HW; all
  tensor-tensor ops keep operand dtypes equal.
"""

import numpy as np

DIM = 512
NH = 4
HD = 128
HD4 = 32
B, H, W = 16, 56, 56
N_CORES = 8
BL = B // N_CORES

TH = 8                  # rows per tile
NT = H // TH            # 7
TN = TH * W             # 448
WP = W + 2              # 58 padded row stride
TPAD = TH * WP          # 464
PADN = (H + 2) * WP     # 3364
NPIX = H * W            # 3136

TAPS = [(dy, dx) for dy in (-1, 0, 1) for dx in (-1, 0, 1)]
NTAP = 9


def default_cfg():
    return dict(
        qkv_fp8=1,
        proj_fp8=1,
        # per-chunk conv2 engine: 'pe' chunks must come in (even, odd)
        # pairs sharing the same engine
        conv2_assign='pe,pe,pe,pe',
        mconv_G=2,
        conv2_G=2,
    )


# layout helpers ------------------------------------------------------------

def _lay_m(oc):
    """channel-within-chunk at partition p of m[oc] (PE 64-block rot)."""
    p = np.arange(HD)
    return 64 * ((p // 64 - oc) % 2) + p % 64


def _lay_id(oc):
    return np.arange(HD)


def layouts(cfg):
    c2a = cfg['conv2_assign'].split(',')
    lay_m = [_lay_m(oc) for oc in range(NH)]          # m buffer layout
    lay_c2 = []                                       # o2 / v layout
    for oc in range(NH):
        if c2a[oc] == 'pe':
            lay_c2.append(_lay_id(oc))                # pair rotation undoes
        else:
            lay_c2.append(lay_m[oc])                  # per-partition engines
    return c2a, lay_m, lay_c2


def build_nc(cfg):
    import concourse.bass as bass
    import concourse.mybir as mybir
    import concourse.tile as tile
    from concourse import bacc
    from contextlib import ExitStack

    f32 = mybir.dt.float32
    bf16 = mybir.dt.bfloat16
    fp8 = mybir.dt.float8e4
    AF = mybir.ActivationFunctionType
    AL = mybir.AluOpType
    DR = mybir.MatmulPerfMode.DoubleRow

    xdt = fp8 if cfg['qkv_fp8'] else bf16
    odt = fp8 if cfg['proj_fp8'] else bf16
    c2a, _, _ = layouts(cfg)
    MG, CG = cfg['mconv_G'], cfg['conv2_G']

    nc = bacc.Bacc("TRN2", target_bir_lowering=False, debug=False,
                   enable_asserts=False, num_devices=N_CORES)

    # ---------------- DRAM I/O ----------------
    x8_d = nc.dram_tensor("x8", [BL, NH, HD, H, W], xdt,
                          kind="ExternalInput").ap()
    xb_d = nc.dram_tensor("xb", [BL, NH, HD, H, W], bf16,
                          kind="ExternalInput").ap()
    out_d = nc.dram_tensor("out", [BL, NH, HD, H, W], bf16,
                           kind="ExternalOutput").ap()
    # consolidated weights: one DMA per group
    wqkv_d = nc.dram_tensor("wqkv", [3, NH, HD, NH, HD], xdt,
                            kind="ExternalInput").ap()
    wp_d = nc.dram_tensor("wp", [NH, HD, NH, HD], odt,
                          kind="ExternalInput").ap()
    dg_d = nc.dram_tensor("dgall", [3, NH, HD, NTAP, 64], bf16,
                          kind="ExternalInput").ap()
    wv2_d = nc.dram_tensor("wvec2", [NH, HD, NTAP], f32,
                           kind="ExternalInput").ap()
    sew1_d = nc.dram_tensor("sew1", [2, NH, HD, HD4], f32,
                            kind="ExternalInput").ap()
    sew2_d = nc.dram_tensor("sew2", [2, NH, 2, HD4, HD], f32,
                            kind="ExternalInput").ap()
    seb1_d = nc.dram_tensor("seb1", [2, NH, HD4], f32,
                            kind="ExternalInput").ap()
    seb2_d = nc.dram_tensor("seb2", [2, NH, 2, HD], f32,
                            kind="ExternalInput").ap()
    bias4_d = nc.dram_tensor("bias4", [4, NH, HD], f32,
                             kind="ExternalInput").ap()

    with tile.TileContext(nc) as tc, ExitStack() as ctx:
        const = ctx.enter_context(tc.tile_pool(name="const", bufs=1))
        big = ctx.enter_context(tc.tile_pool(name="big", bufs=1))
        st = ctx.enter_context(tc.tile_pool(name="st", bufs=2))
        pp = ctx.enter_context(tc.tile_pool(name="pp", bufs=8, space="PSUM"))

        qpad = [big.tile([HD, PADN + 2], bf16, name=f"qpad{c}") for c in range(NH)]
        kpad = [big.tile([HD, PADN + 2], bf16, name=f"kpad{c}") for c in range(NH)]
        mpad = [big.tile([HD, PADN + 2], bf16, name=f"mpad{c}") for c in range(NH)]
        vsb = [big.tile([HD, NPIX], bf16, name=f"vsb{c}") for c in range(NH)]
        o2all = big.tile([HD, NH, NPIX], odt, name="o2all")
        acc = [big.tile([HD, H * WP], bf16, name=f"acc{i}") for i in range(3)]

        def pad3(t):
            return t[:, 1:1 + PADN].rearrange("p (h w) -> p h w", w=WP)

        # zero pad cells: top row(+slop), bottom row(+slop), col pads
        for t in qpad + kpad + mpad:
            nc.vector.memset(t[:, 0:1 + WP + 1], 0.0)
            nc.vector.memset(t[:, 1 + (H + 1) * WP:PADN + 2], 0.0)
            nc.vector.memset(
                t[:, 1 + WP + W + 1:1 + WP + W + 1 + H * WP].rearrange(
                    "p (h w) -> p h w", w=WP)[:, :, 0:2], 0.0)

        # ---------- weights (one DMA per group, spread over queues) ----------
        wqkv_sb = const.tile([HD, 3, NH, NH, HD], xdt, name="wqkv_sb")
        nc.sync.dma_start(wqkv_sb, wqkv_d.rearrange("b o p k m -> p b o k m"))
        wp_sb = const.tile([HD, NH, NH, HD], odt, name="wp_sb")
        nc.gpsimd.dma_start(wp_sb, wp_d.rearrange("o p k m -> p o k m"))
        gemm_w = {}
        for bi, nm in enumerate(('q', 'k', 'v')):
            for oc in range(NH):
                gemm_w[nm, oc] = wqkv_sb[:, bi, oc]
        for oc in range(NH):
            gemm_w['p', oc] = wp_sb[:, oc]
        dg_sb = const.tile([HD, 3, NH, NTAP, 64], bf16, name="dg_sb")
        nc.scalar.dma_start(dg_sb, dg_d.rearrange("b o p j e -> p b o j e"))
        dg = {}
        for bi, key in enumerate(('q', 'k', '2')):
            for oc in range(NH):
                dg[key, oc] = dg_sb[:, bi, oc]
        dgs = {}
        for key in ('q', 'k'):
            for oc in range(NH):
                dgs[key, oc] = const.tile([HD, NTAP, 64], bf16,
                                          name=f"dgs{key}{oc}")
        wvec2_sb = const.tile([HD, NH, NTAP], f32, name="wvec2_sb")
        nc.gpsimd.dma_start(wvec2_sb, wv2_d.rearrange("o p j -> p o j"))
        wvec2 = {oc: wvec2_sb[:, oc] for oc in range(NH)}

        sew1_sb = const.tile([HD, 2, NH, HD4], f32, name="sew1_sb")
        nc.scalar.dma_start(sew1_sb, sew1_d.rearrange("b o p f -> p b o f"))
        sew2_sb = const.tile([HD4, 2, NH, 2, HD], f32, name="sew2_sb")
        nc.scalar.dma_start(sew2_sb, sew2_d.rearrange("b o k p m -> p b o k m"))
        seb1_sb = const.tile([HD4, 2, NH], f32, name="seb1_sb")
        nc.gpsimd.dma_start(seb1_sb, seb1_d.rearrange("b o p -> p b o"))
        seb2_sb = const.tile([HD, 2, NH, 2], f32, name="seb2_sb")
        nc.gpsimd.dma_start(seb2_sb, seb2_d.rearrange("b o k p -> p b o k"))
        bias4_sb = const.tile([HD, 4, NH], f32, name="bias4_sb")
        nc.scalar.dma_start(bias4_sb, bias4_d.rearrange("k o p -> p k o"))
        se = {}
        for br in range(2):
            for oc in range(NH):
                se['w1', br, oc] = sew1_sb[:, br, oc]
                se['b1', br, oc] = seb1_sb[:, br, oc:oc + 1]
                se['w2n', br, oc] = sew2_sb[:, br, oc, 0]
                se['w2r', br, oc] = sew2_sb[:, br, oc, 1]
                se['b2n', br, oc] = seb2_sb[:, br, oc, 0:1]
                se['b2r', br, oc] = seb2_sb[:, br, oc, 1:2]
        bqr = [bias4_sb[:, 0, oc:oc + 1] for oc in range(NH)]
        bkr = [bias4_sb[:, 1, oc:oc + 1] for oc in range(NH)]
        dwcb = [bias4_sb[:, 2, oc:oc + 1] for oc in range(NH)]
        projb = [bias4_sb[:, 3, oc:oc + 1] for oc in range(NH)]

        def tap_sl(buf, t, j):
            dy, dx = TAPS[j]
            base = 1 + (t * TH + 1) * WP + dy * WP + dx
            return buf[:, base:base + TPAD]

        def gemm(ps, wtile, xtile, tsl, dr):
            if dr:
                for ks in range(0, NH, 2):
                    nc.tensor.matmul(ps, wtile[:, ks:ks + 2, :],
                                     xtile[:, ks:ks + 2, tsl],
                                     start=(ks == 0), stop=(ks == 2),
                                     perf_mode=DR)
            else:
                for kc in range(NH):
                    nc.tensor.matmul(ps, wtile[:, kc, :], xtile[:, kc, tsl],
                                     start=(kc == 0), stop=(kc == NH - 1))

        # ================= per-sample body =================
        def emit_sample(b):
            stats = st.tile([HD, 2 * NH * NT], f32, name=f"stats{b}",
                            tag="stats")
            # ---- PH_A: q,k GEMM + fused pooling ----
            for t in range(NT):
                r0 = t * TH
                xt = st.tile([HD, NH, TN], xdt, tag="xt", bufs=3,
                             name=f"xt{b}_{t}")
                nc.sync.dma_start(
                    xt.rearrange("p c (h w) -> p c h w", w=W),
                    x8_d[b, :, :, r0:r0 + TH, :].rearrange(
                        "c p h w -> p c h w"))
                for br, (nm, dst) in enumerate((('q', qpad), ('k', kpad))):
                    for oc in range(NH):
                        ps = pp.tile([HD, TN], f32, tag="ps",
                                     name=f"g{b}_{br}_{t}_{oc}")
                        gemm(ps, gemm_w[nm, oc], xt, slice(None),
                             cfg['qkv_fp8'])
                        nc.scalar.activation(
                            pad3(dst[oc])[:, 1 + r0:1 + r0 + TH, 1:1 + W],
                            ps.rearrange("p (h w) -> p h w", w=W),
                            AF.Identity,
                            accum_out=stats[:, (br * NH + oc) * NT + t:
                                            (br * NH + oc) * NT + t + 1])
            # ---- V GEMM (covers SE latency) ----
            for t in range(NT):
                r0 = t * TH
                xt = st.tile([HD, NH, TN], xdt, tag="xt", bufs=3,
                             name=f"xtv{b}_{t}")
                nc.sync.dma_start(
                    xt.rearrange("p c (h w) -> p c h w", w=W),
                    x8_d[b, :, :, r0:r0 + TH, :].rearrange(
                        "c p h w -> p c h w"))
                for oc in range(NH):
                    ps = pp.tile([HD, TN], f32, tag="ps", name=f"v{b}_{t}_{oc}")
                    gemm(ps, gemm_w['v', oc], xt, slice(None), cfg['qkv_fp8'])
                    nc.scalar.copy(vsb[oc][:, t * TN:(t + 1) * TN], ps)
            # ---- SE ----
            s_rot = [[None] * NH for _ in range(2)]
            for br in range(2):
                for oc in range(NH):
                    pooled = const.tile([HD, 1], f32, tag="pooled", bufs=4,
                                        name=f"pool{b}_{br}_{oc}")
                    i0 = (br * NH + oc) * NT
                    nc.vector.tensor_reduce(pooled, stats[:, i0:i0 + NT],
                                            mybir.AxisListType.X, AL.add)
                    ps1 = pp.tile([HD4, 1], f32, tag="ps",
                                  name=f"se1_{b}_{br}_{oc}")
                    nc.tensor.matmul(ps1, se['w1', br, oc], pooled,
                                     start=True, stop=True)
                    hvec = const.tile([HD4, 1], f32, tag="hvec", bufs=4,
                                      name=f"h{b}_{br}_{oc}")
                    nc.scalar.activation(hvec, ps1, AF.Relu,
                                         bias=se['b1', br, oc])
                    ps2 = pp.tile([HD, 1], f32, tag="ps",
                                  name=f"se2_{b}_{br}_{oc}")
                    nc.tensor.matmul(ps2, se['w2n', br, oc], hvec,
                                     start=True, stop=True)
                    s_nat = const.tile([HD, 1], f32, tag="s_nat", bufs=8,
                                       name=f"sn{b}_{br}_{oc}")
                    nc.scalar.activation(s_nat, ps2, AF.Sigmoid,
                                         bias=se['b2n', br, oc])
                    ps2r = pp.tile([HD, 1], f32, tag="ps",
                                   name=f"se2r_{b}_{br}_{oc}")
                    nc.tensor.matmul(ps2r, se['w2r', br, oc], hvec,
                                     start=True, stop=True)
                    sr = const.tile([HD, 1], f32, tag="s_rot", bufs=8,
                                    name=f"sr{b}_{br}_{oc}")
                    nc.scalar.activation(sr, ps2r, AF.Sigmoid,
                                         bias=se['b2r', br, oc])
                    s_rot[br][oc] = sr
                    key = 'q' if br == 0 else 'k'
                    nc.vector.tensor_scalar(dgs[key, oc], dg[key, oc], s_nat,
                                            None, AL.mult)
            bias_m = []
            for oc in range(NH):
                tmp = const.tile([HD, 1], f32, tag="bm_tmp", bufs=2,
                                 name=f"bmt{b}_{oc}")
                nc.vector.tensor_scalar(tmp, bqr[oc], s_rot[0][oc],
                                        None, AL.mult)
                bm = const.tile([HD, 1], f32, tag="bias_m", bufs=4,
                                name=f"bm{b}_{oc}")
                nc.vector.scalar_tensor_tensor(bm, bkr[oc], s_rot[1][oc], tmp,
                                               AL.mult, AL.add)
                bias_m.append(bm)

            # ---- m-conv on PE: 18 taps, 64x64 blocks, pair rotation ----
            for g in range(0, NT, MG):
                gts = list(range(g, min(g + MG, NT)))
                pst = [[pp.tile([HD, TPAD], f32, tag="ps",
                                name=f"m{b}_{t}_{oc}") for oc in range(NH)]
                       for t in gts]
                for jj in range(2 * NTAP):
                    br, j = divmod(jj, NTAP)
                    key, src = ('q', qpad) if br == 0 else ('k', kpad)
                    for pair in range(2):
                        for cc in range(2):
                            oc = 2 * pair + cc
                            for a in range(2):
                                bb = (a + cc) % 2
                                for ti in range(len(gts)):
                                    nc.tensor.matmul(
                                        pst[ti][oc][64 * bb:64 * bb + 64, :],
                                        dgs[key, oc][64 * a:64 * a + 64, j, :],
                                        tap_sl(src[oc], gts[ti], j)[
                                            64 * a:64 * a + 64, :],
                                        start=(jj == 0),
                                        stop=(jj == 2 * NTAP - 1),
                                        tile_position=(64 * a, 64 * bb),
                                        skip_group_check=True)
                for ti, t in enumerate(gts):
                    for oc in range(NH):
                        nc.scalar.activation(
                            pad3(mpad[oc])[:, 1 + t * TH:1 + t * TH + TH,
                                           1:1 + W],
                            pst[ti][oc].rearrange(
                                "p (h w) -> p h w", w=WP)[:, :, 1:1 + W],
                            AF.Identity, bias=bias_m[oc])

            # ---- conv2 per chunk on assigned engine, then o2 = (c2+b)*v ----
            pe_ocs = [oc for oc in range(NH) if c2a[oc] == 'pe']
            for g in range(0, NT, CG):
                gts = list(range(g, min(g + CG, NT)))
                if pe_ocs:
                    pst = [{oc: pp.tile([HD, TPAD], f32, tag="ps",
                                        name=f"c2{b}_{t}_{oc}")
                            for oc in pe_ocs} for t in gts]
                    for j in range(NTAP):
                        for oc in pe_ocs:
                            cc = oc % 2
                            for bb in range(2):
                                b2 = (bb + cc) % 2
                                for ti in range(len(gts)):
                                    nc.tensor.matmul(
                                        pst[ti][oc][64 * b2:64 * b2 + 64, :],
                                        dg['2', oc][64 * bb:64 * bb + 64, j, :],
                                        tap_sl(mpad[oc], gts[ti], j)[
                                            64 * bb:64 * bb + 64, :],
                                        start=(j == 0), stop=(j == NTAP - 1),
                                        tile_position=(64 * bb, 64 * b2),
                                        skip_group_check=True)
                    for ti, t in enumerate(gts):
                        for oc in pe_ocs:
                            c2t = st.tile([HD, TN], bf16, tag="c2t", bufs=4,
                                          name=f"c2t{b}_{t}_{oc}")
                            nc.scalar.activation(
                                c2t.rearrange("p (h w) -> p h w", w=W),
                                pst[ti][oc].rearrange(
                                    "p (h w) -> p h w", w=WP)[:, :, 1:1 + W],
                                AF.Identity, bias=dwcb[oc])
                            nc.vector.tensor_tensor(
                                o2all[:, oc, t * TN:(t + 1) * TN], c2t,
                                vsb[oc][:, t * TN:(t + 1) * TN], AL.mult)
            # dve / dva chunks: whole-image tap chains in SBUF
            for oc in range(NH):
                eng = c2a[oc]
                if eng == 'pe':
                    continue
                wv2 = wvec2[oc]
                cur, nxt, tmp = 0, 1, 2
                dy, dx = TAPS[0]
                base = 1 + WP + dy * WP + dx      # rows 1..H, all cols
                cnt = H * WP
                nc.vector.tensor_scalar(
                    acc[cur][:, 0:cnt],
                    mpad[oc][:, base:base + cnt], wv2[:, 0:1], None, AL.mult)
                for j in range(1, NTAP):
                    dy, dx = TAPS[j]
                    base = 1 + WP + dy * WP + dx
                    msl = mpad[oc][:, base:base + cnt]
                    if eng == 'dve':
                        nc.vector.tensor_scalar(
                            acc[tmp][:, 0:cnt], msl, wv2[:, j:j + 1],
                            None, AL.mult)
                        nc.vector.tensor_tensor(
                            acc[nxt][:, 0:cnt], acc[cur][:, 0:cnt],
                            acc[tmp][:, 0:cnt], AL.add)
                    else:  # dva: ACT scale-copy + Pool accumulate
                        nc.scalar.activation(
                            acc[tmp][:, 0:cnt], msl, AF.Copy,
                            scale=wv2[:, j:j + 1])
                        nc.gpsimd.tensor_tensor(
                            acc[nxt][:, 0:cnt], acc[cur][:, 0:cnt],
                            acc[tmp][:, 0:cnt], AL.add)
                    cur, nxt, tmp = nxt, tmp, cur
                # o2 = (acc + dwc_b) * v  -- acc rows 1..H, interior cols
                acc3 = acc[cur][:, 0:cnt].rearrange("p (h w) -> p h w", w=WP)
                nc.vector.scalar_tensor_tensor(
                    o2all[:, oc, :].rearrange("p (h w) -> p h w", w=W),
                    acc3[:, :, 1:1 + W], dwcb[oc],
                    vsb[oc].rearrange("p (h w) -> p h w", w=W),
                    AL.add, AL.mult)

            # ---- proj GEMM + residual + out ----
            for t in range(NT):
                r0 = t * TH
                xb = st.tile([HD, NH, TN], bf16, tag="xb", bufs=2,
                             name=f"xb{b}_{t}")
                nc.sync.dma_start(
                    xb.rearrange("p c (h w) -> p c h w", w=W),
                    xb_d[b, :, :, r0:r0 + TH, :].rearrange(
                        "c p h w -> p c h w"))
                ot = st.tile([HD, NH, TN], bf16, tag="ot", bufs=2,
                             name=f"ot{b}_{t}")
                for oc in range(NH):
                    ps = pp.tile([HD, TN], f32, tag="ps", name=f"p{b}_{t}_{oc}")
                    gemm(ps, gemm_w['p', oc],
                         o2all.rearrange("p c n -> p c n"),
                         slice(t * TN, (t + 1) * TN), cfg['proj_fp8'])
                    pt = st.tile([HD, TN], bf16, tag="pt", bufs=4,
                                 name=f"pt{b}_{t}_{oc}")
                    nc.scalar.activation(pt, ps, AF.Identity, bias=projb[oc])
                    nc.vector.tensor_tensor(ot[:, oc, :], pt, xb[:, oc, :],
                                            AL.add)
                nc.sync.dma_start(
                    out_d[b, :, :, r0:r0 + TH, :].rearrange(
                        "c p h w -> p c h w"),
                    ot.rearrange("p c (h w) -> p c h w", w=W))

        for b in range(BL):
            emit_sample(b)

    nc.compile()
    return nc


# ---------------------------------------------------------------------------
# host-side weight prep
# ---------------------------------------------------------------------------

def prep_weights(inputs, cfg):
    import ml_dtypes
    f32 = np.float32
    bf = ml_dtypes.bfloat16
    e4 = ml_dtypes.float8_e4m3
    xdt = e4 if cfg['qkv_fp8'] else bf
    odt = e4 if cfg['proj_fp8'] else bf
    c2a, lay_m, lay_c2 = layouts(cfg)

    qkv_w = np.asarray(inputs['qkv_w'], f32)
    proj_w = np.asarray(inputs['proj_w'], f32)

    def gemm_blocks(wmat, row_perm, dt):
        # lhsT blocks [oc][k_part, k_sub, m]; row_perm permutes output chans
        out = np.empty((NH, HD, NH, HD), f32)
        for oc in range(NH):
            rows = wmat[oc * HD:(oc + 1) * HD]
            if row_perm is not None:
                rows = rows[row_perm[oc]]
            out[oc] = rows.reshape(HD, NH, HD).transpose(2, 1, 0)
        return np.ascontiguousarray(out).astype(dt)

    def proj_blocks(wmat):
        # input (o2) channels are in lay_c2 layout: permute columns
        out = np.empty((NH, HD, NH, HD), f32)
        for oc in range(NH):
            cols = wmat[oc * HD:(oc + 1) * HD].reshape(HD, NH, HD)
            perm_cols = np.empty_like(cols)
            for kc in range(NH):
                perm_cols[:, kc, :] = cols[:, kc, lay_c2[kc]]
            out[oc] = perm_cols.transpose(2, 1, 0)
        return np.ascontiguousarray(out).astype(odt)

    def diag_blocks(wconv, row_perm=None):
        w = np.asarray(wconv, f32).reshape(DIM, NTAP)
        out = np.zeros((NH, HD, NTAP, 64), f32)
        for oc in range(NH):
            ch = np.arange(HD) if row_perm is None else row_perm[oc]
            for p in range(HD):
                out[oc, p, :, p % 64] = w[oc * HD + ch[p]]
        return out.astype(bf)

    def wvec_l(wconv):
        # per-partition conv2 weights in the m layout
        w = np.asarray(wconv, f32).reshape(DIM, NTAP)
        out = np.empty((NH, HD, NTAP), f32)
        for oc in range(NH):
            out[oc] = w[oc * HD + lay_m[oc]]
        return out

    npix = float(NPIX)
    wsum_q = np.asarray(inputs['sq_w'], f32).reshape(DIM, NTAP).sum(1)
    wsum_k = np.asarray(inputs['sk_w'], f32).reshape(DIM, NTAP).sum(1)

    def se_prep(w1, b1, w2, b2, wsum, cbias):
        w1 = np.asarray(w1, f32)
        b1 = np.asarray(b1, f32)
        w2 = np.asarray(w2, f32)
        b2 = np.asarray(b2, f32)
        cbias = np.asarray(cbias, f32).reshape(NH, HD)
        sew1 = np.empty((NH, HD, HD4), f32)
        seb1 = np.empty((NH, HD4, 1), f32)
        sew2n = np.empty((NH, HD4, HD), f32)
        seb2n = np.asarray(b2, f32).reshape(NH, HD, 1).copy()
        sew2r = np.empty((NH, HD4, HD), f32)
        seb2r = np.empty((NH, HD, 1), f32)
        for oc in range(NH):
            sew1[oc] = (w1[oc] * (wsum[oc * HD:(oc + 1) * HD] / npix)[None, :]).T
            seb1[oc] = (b1[oc] + w1[oc] @ cbias[oc]).reshape(HD4, 1)
            sew2n[oc] = w2[oc].T
            r = lay_m[oc]
            sew2r[oc] = w2[oc][r].T
            seb2r[oc] = b2[oc][r].reshape(HD, 1)
        return dict(w1=sew1, b1=seb1, w2n=sew2n, b2n=seb2n, w2r=sew2r,
                    b2r=seb2r)

    sq_b = np.asarray(inputs['sq_b'], f32)
    sk_b = np.asarray(inputs['sk_b'], f32)
    dwc_b = np.asarray(inputs['dwc_b'], f32)
    se_q = se_prep(inputs['cq_w1'], inputs['cq_b1'], inputs['cq_w2'],
                   inputs['cq_b2'], wsum_q, sq_b)
    se_k = se_prep(inputs['ck_w1'], inputs['ck_b1'], inputs['ck_w2'],
                   inputs['ck_b2'], wsum_k, sk_b)

    bias4 = np.empty((4, NH, HD), f32)
    for oc in range(NH):
        bias4[0, oc] = sq_b[oc * HD + lay_m[oc]]
        bias4[1, oc] = sk_b[oc * HD + lay_m[oc]]
        # conv2 'pe' chunks add dwc_b at the (natural-layout) drain;
        # dve/dva chunks add it inside the STT in m layout
        perm = np.arange(HD) if c2a[oc] == 'pe' else lay_m[oc]
        bias4[2, oc] = dwc_b[oc * HD + perm]
    bias4[3] = np.asarray(inputs['proj_b'], f32).reshape(NH, HD)

    wqkv = np.stack([
        gemm_blocks(qkv_w[0:DIM], None, f32),
        gemm_blocks(qkv_w[DIM:2 * DIM], None, f32),
        gemm_blocks(qkv_w[2 * DIM:3 * DIM], lay_c2, f32)]).astype(xdt)
    dgall = np.stack([
        diag_blocks(inputs['sq_w']),
        diag_blocks(inputs['sk_w']),
        diag_blocks(inputs['dwc_w'], row_perm=lay_m)])
    sew1 = np.stack([se_q['w1'], se_k['w1']])             # [2,NH,HD,HD4]
    sew2 = np.stack([
        np.stack([se_q['w2n'], se_q['w2r']], axis=1),
        np.stack([se_k['w2n'], se_k['w2r']], axis=1)])    # [2,NH,2,HD4,HD]
    seb1 = np.stack([se_q['b1'], se_k['b1']])[..., 0]     # [2,NH,HD4]
    seb2 = np.stack([
        np.stack([se_q['b2n'], se_q['b2r']], axis=1),
        np.stack([se_k['b2n'], se_k['b2r']], axis=1)])[..., 0]  # [2,NH,2,HD]

    return dict(
        wqkv=wqkv,
        wp=proj_blocks(proj_w),
        dgall=dgall,
        wvec2=wvec_l(inputs['dwc_w']),
        sew1=sew1, sew2=sew2, seb1=seb1, seb2=seb2, bias4=bias4,
    )


_CACHE = {}


def _get_compiled(cfg_key, cfg):
    if cfg_key not in _CACHE:
        _CACHE[cfg_key] = build_nc(cfg)
    return _CACHE[cfg_key]


def make_in_maps(inputs, cfg):
    import ml_dtypes
    w = prep_weights(inputs, cfg)
    x32 = np.asarray(inputs['x'], np.float32).reshape(B, NH, HD, H, W)
    xdt = ml_dtypes.float8_e4m3 if cfg['qkv_fp8'] else ml_dtypes.bfloat16
    x8 = np.ascontiguousarray(x32).astype(xdt)
    xbf = np.ascontiguousarray(x32).astype(ml_dtypes.bfloat16)
    in_maps = []
    for core in range(N_CORES):
        m = dict(w)
        m['x8'] = x8[core * BL:(core + 1) * BL]
        m['xb'] = xbf[core * BL:(core + 1) * BL]
        in_maps.append(m)
    return in_maps


def gather_out(results):
    out = np.empty((B, DIM, H, W), np.float32)
    for core in range(N_CORES):
        out[core * BL:(core + 1) * BL] = np.asarray(
            results[core]['out'], np.float32).reshape(BL, DIM, H, W)
    return out


def kernel(**inputs):
    from concourse import bass_utils
    cfg = default_cfg()
    nc = _get_compiled('main', cfg)
    in_maps = make_in_maps(inputs, cfg)
    res = bass_utils.run_bass_kernel_spmd(nc, in_maps,
                                          core_ids=list(range(N_CORES)))
    return gather_out(res.results)
